# revision 57
# baseline (speedup 1.0000x reference)
"""Trainium2 Bass kernel for the BDH-style recurrent block.

Strategy: data-parallel over B (8 batches -> 8 NeuronCores, no collectives).
The T=128-step scan is de-sequentialized into dense matmuls per core:

  u_t = relu(emb_t @ Dx.T)                                  (T,N)
  x_t = (XD*x_{t-1} + u_t)/s_t  with s_t = XD + sum(u_t)    (L1 norm; x>=0)
      => X = C @ u, C[t,s] = (1/c_s) exp(A_t - A_s), A_t = cumsum log(XD/c_r)
  a*_t = rho_{t-1} @ x_t = ((DecayMask . X X^T) @ ln(emb))_t   (rho_0 = 0)
  y_t  = relu(ln(a*_t) @ Dy.T) * x_t                        (x_t >= 0)
  v*_t = ln(y_t @ E.T)

All matmuls run in bf16 (1 col/cycle at any free dim, vs f32r's 4 cyc/row
below 256 free): X^T and Ycore^T are produced directly in n-major chunks
(lhsT = u chunk / DyT chunk), so no PE transposes of X/Y are needed.
LayerNorm rstd uses exp(-0.5*ln(v+eps)) so the only ACT table set ever
needed is natural_log_exp_and_others -> one table load, at kernel start.
Inputs arrive as one packed bf16 blob + a small f32 const tensor, posted
in need-order across the Sync and Scalar HWDGE queues.
"""

import math
from contextlib import ExitStack

import numpy as np

N = 2048
D = 256
B = 8
T = 128
XD = 0.97
UD = 0.97
LN_EPS = 1e-5

# log-domain recentring: E[sum relu(N(0,1)) over 2048] + XD ~ 818.9
LNC2INV = 6.7065
C2 = math.exp(-LNC2INV)
K1 = LNC2INV - math.log(XD)

KD = D // 128    # 2
KN = N // 128    # 16
NJ = N // 512    # 4
WARMUP_MMS = 16  # 256-col bf16 MMs while the first DMA piece streams

# bf16 blob column layout (per 128-partition row), in DMA need-order:
# embT+dxT stream first (u is the critical path), consts/emb can wait
# until the qc/vn stage, then dyT/eT for the late phases.
C_EMBT = 0               # embT (KD*T)
C_DX = C_EMBT + KD * T   # dxT packed [j(4), k(2), 512]
C_US = C_DX + KD * N     # ustrict [s< t]
C_DM = C_US + T          # dmaskT  UD^(t-1-s)
C_EMB = C_DM + T         # emb  (D)
C_DY = C_EMB + D         # dyT packed [c(16), k(2), 128]
C_ET = C_DY + KD * N     # eT  packed [c(16), 256]
CB = C_ET + KN * D


def _consts_bf16():
    import ml_dtypes
    r = np.arange(T)
    ustrict = (r[:, None] < r[None, :]).astype(np.float32)
    pw = r[:, None] - 1 - r[None, :]                        # [t,s] t-1-s
    dmask = np.where(pw >= 0, UD ** np.maximum(pw, 0), 0.0).astype(np.float32)
    dmaskT = np.ascontiguousarray(dmask.T)                  # [s,t]
    return np.concatenate([ustrict, dmaskT],
                          axis=1).astype(ml_dtypes.bfloat16)


def _consts_f32():
    r = np.arange(T)
    tri = r[None, :] - r[:, None]                           # t - s
    trik = np.where(tri >= 0, -K1 * tri - LNC2INV, -10000.0)
    trik = np.maximum(trik, -85.0).astype(np.float32)
    xdvec = np.full((T, 1), C2 * XD, dtype=np.float32)
    xdvec[0, 0] = 0.0                                       # x_{-1} = 0
    zero = np.zeros((T, 1), dtype=np.float32)
    return np.ascontiguousarray(np.concatenate([trik, xdvec, zero], axis=1))


_cache = {}


def _split_multiwait(nc, mybir):
    """This walrus build caps sync waits per instruction (1 for regular
    instructions, 2 for EventSemaphore). Tile attaches more (e.g. the
    kernel-tail Drain waits on every live semaphore). Hoist excess waits
    onto same-engine NOPs placed immediately before the instruction —
    engine queues are sequential, so semantics are preserved."""
    n = 0
    for f in nc.m.functions:
        for bb in f.blocks:
            out = []
            changed = False
            for ins in bb.instructions:
                si = ins.sync_info
                ow = list(si.on_wait) if si is not None else []
                cap = 2 if ins.opcode == "EventSemaphore" else 1
                if len(ow) > cap:
                    sem_waits = [w for w in ow if w.sync_type == "semaphore"]
                    other = [w for w in ow if w.sync_type != "semaphore"]
                    keep = max(cap - len(other), 0)
                    hoist = sem_waits[:len(sem_waits) - keep] if keep else sem_waits
                    kept = sem_waits[len(hoist):] + other
                    assert len(kept) <= cap, (len(kept), cap, ins.opcode)
                    changed = True
                    for w in hoist:
                        n += 1
                        nop = mybir.InstNoOp(
                            name=f"wsplit-{n}",
                            sync_info=mybir.SyncInfo(on_wait=[w], on_update=[]),
                            bass_nofuse=True,
                            engine=ins.engine,
                        )
                        nc.register_instruction(nop, overwrite=True)
                        out.append(nop)
                    si.on_wait = kept
                out.append(ins)
            if changed:
                bb.instructions = out
    return nc


def _build():
    import concourse.bass as bass
    import concourse.mybir as mybir
    import concourse.tile as tile

    f32 = mybir.dt.float32
    bf16 = mybir.dt.bfloat16
    AF = mybir.ActivationFunctionType
    ALU = mybir.AluOpType
    AX = mybir.AxisListType

    from concourse.vector_clock import ScopedClock

    class _TrimTailTC(tile.TileContext):
        # Drop the second kernel-tail all-engine barrier: it only orders
        # the semaphore resets against engine halt, and nothing executes
        # after it. The first barrier (before resets) is kept, so resets
        # still happen on a quiesced machine and re-execution stays safe.
        def _drain_and_barrier(self, tick_clock, wait_clock):
            drain_inst = self.nc.sync.drain()
            wait_clock.add_sem_waits(
                drain_inst.ins, ScopedClock({None: tick_clock.global_clock})
            )
            self.nc.all_engine_barrier()
            assert self.sems is not None
            popped = self.nc._tile_sem_poison_stack.pop()
            assert popped is self._sem_poison
            self.nc.clear_and_free_semaphores(
                list(self.sems.allocated().values())
            )

    nc = bass.Bass()

    d_blob = nc.dram_tensor("blob", [128, CB], bf16, kind="ExternalInput")
    d_cf32 = nc.dram_tensor("cf32", [128, T + 2], f32, kind="ExternalInput")
    d_out = nc.dram_tensor("out", [T, D], bf16, kind="ExternalOutput")

    with _TrimTailTC(nc) as tc, ExitStack() as ctx:
        work = ctx.enter_context(tc.tile_pool(name="work", bufs=1))
        stats = ctx.enter_context(tc.tile_pool(name="stats", bufs=1))
        # PSUM slots are bank-granular: 8 banks of [128,512]xf32 total.
        # Tags: pu(2) warmup/u-phase, ch(4) xt/tpb/yc rotation,
        # g(1) g->aps->vps rotation, cc(1) qc/keepalive rotation.
        p_ps = ctx.enter_context(tc.tile_pool(name="p_ps", bufs=1, space="PSUM"))

        # ---- DMAs first: HWDGE FIFO order per queue = need order. -------
        blob_sb = work.tile([128, CB], bf16)
        cf32_sb = work.tile([128, T + 2], f32)
        # Seven pieces. Completion receipts are processed BY THE POSTING
        # ENGINE between its instructions, so receipts serialize behind any
        # still-queued posts (measured: 10 posts pushed piece-0's receipt
        # from 10.7us to 12.7us). Posts finish by ~12us, and the per-j
        # dxT splits release each u chunk ~1.3us after its bytes land.
        cuts = [0, C_DX + 1024, C_DX + 2048, C_DX + 3072, C_US, C_DY,
                C_ET, CB]
        for a, b in zip(cuts[:-1], cuts[1:]):
            nc.sync.dma_start(blob_sb[:, a:b], d_blob[:, a:b])
        nc.scalar.dma_start(cf32_sb[:], d_cf32[:])

        ustrict_sb = blob_sb[:, C_US:C_US + T]
        dmaskT_sb = blob_sb[:, C_DM:C_DM + T]
        embT_sb = blob_sb[:, C_EMBT:C_EMBT + KD * T]
        emb_sb = blob_sb[:, C_EMB:C_EMB + D]
        dxT_sb = blob_sb[:, C_DX:C_DX + KD * N]
        dyT_sb = blob_sb[:, C_DY:C_DY + KD * N]
        eT_sb = blob_sb[:, C_ET:C_ET + KN * D]
        trik_sb = cf32_sb[:, 0:T]
        xdvec_sb = cf32_sb[:, T:T + 1]
        zero_sb = cf32_sb[:, T + 1:T + 2]

        # ---- ACT table preload: Ln+Exp share one set; everything else the
        # kernel uses (Relu/Copy/Identity) is a filler in every set.
        pre_o = stats.tile([1, 1], f32)
        nc.scalar.activation(pre_o[:], zero_sb[0:1, :], AF.Ln,
                             bias=zero_sb[0:1, :])
        nc.scalar.activation(pre_o[:], zero_sb[0:1, :], AF.Exp,
                             bias=zero_sb[0:1, :])

        # ---- PE warmup: random-data bf16 matmuls while inputs stream ----
        # (all-zero operands leave the HAM activity monitor cold)
        # Warmup reads uninitialized SBUF: junk only feeds discarded
        # warmup/keepalive matmuls, never the output datapath.
        wu_t = nc.alloc_sbuf_tensor("wu_raw", [128, 256], bf16)
        wu_sb = wu_t.ap()
        wu_ps = p_ps.tile([128, 512], f32, tag="pu", bufs=3)
        for _ in range(WARMUP_MMS):
            nc.tensor.matmul(wu_ps[:, 0:256], wu_sb[:, 0:128], wu_sb[:],
                             start=True, stop=True)

        def kdense(n=1):
            # Ungated full-width junk matmuls: queue in PE program order
            # and stream at ~100% duty, filling idle PE stretches so the
            # HAM activity window never judges the PE idle (threshold is
            # high: ~60% array duty; real phases here sit at 35-50%).
            # They ACCUMULATE into one garbage tile: accumulating matmuls
            # to the same bank pipeline (~110ns), while separate start/stop
            # groups pay a ~325ns WAW drain each.
            kd = p_ps.tile([128, 256], f32, tag="cc", bufs=1)
            for i in range(n):
                nc.tensor.matmul(kd[:], embT_sb[:, 0:128], wu_sb[:],
                                 start=(i == 0), stop=(i == n - 1))

        def keepalive(ap_bf, dense=2):
            # One matmul gated on the late chain value ap_bf (bf16 [T,1]
            # lhsT) orders the following dense junk after it.
            ka = p_ps.tile([1, 256], f32, tag="cc", bufs=1)
            nc.tensor.matmul(ka[:], ap_bf, wu_sb[:], start=True, stop=True)
            kdense(dense)

        def ln_stats(src, tagp, nq=None, cinv2=None):
            """scale/nmr for LN over the free dim. rstd = exp(-0.5*ln(v+eps))
            keeps everything in the natural_log_exp table set. When the rows
            of src carry a known positive scale c (cinv=1/c, cinv2=1/c^2),
            the stats are corrected so eps applies to the TRUE variance —
            LN is only scale-invariant when v >> eps, which fails for the
            near-zero early rows of a*."""
            stat6 = stats.tile([T, 6], f32, tag=f"{tagp}_s6")
            nc.vector.bn_stats(stat6[:], src)
            mv = stats.tile([T, 2], f32, tag=f"{tagp}_mv")
            nc.vector.bn_aggr(mv[:], stat6[:])
            veps = stats.tile([T, 1], f32, tag=f"{tagp}_ve")
            if cinv2 is None:
                nc.vector.tensor_scalar_add(veps[:], mv[:, 1:2], LN_EPS)
            else:
                nc.vector.tensor_scalar(veps[:], mv[:, 1:2], cinv2[:], LN_EPS,
                                        op0=ALU.mult, op1=ALU.add)
            lv = stats.tile([T, 1], f32, tag=f"{tagp}_lv")
            nc.scalar.activation(lv[:], veps[:], AF.Ln, bias=zero_sb)
            scl = stats.tile([T, 1], f32, tag=f"{tagp}_sc")
            nc.scalar.activation(scl[:], lv[:], AF.Exp, scale=-0.5,
                                 bias=(zero_sb if nq is None else nq[:]))
            nmr = stats.tile([T, 1], f32, tag=f"{tagp}_nr")
            nc.vector.scalar_tensor_tensor(nmr[:], mv[:, 0:1], -1.0, scl[:],
                                           op0=ALU.mult, op1=ALU.mult)
            return scl, nmr

        # ---- u = relu(emb @ Dx.T) (bf16), row sums --------------------
        u_sb = work.tile([T, N], bf16)
        su_part = stats.tile([T, NJ], f32)
        for j in range(NJ):
            ps = p_ps.tile([128, 512], f32, tag="pu", bufs=3)
            for k in range(KD):
                nc.tensor.matmul(
                    ps[:],
                    embT_sb[:, k * T:(k + 1) * T],
                    dxT_sb[:, j * 1024 + k * 512: j * 1024 + (k + 1) * 512],
                    start=(k == 0),
                    stop=(k == KD - 1),
                )
            if j % 2 == 0:
                nc.scalar.activation(
                    u_sb[:, j * 512:(j + 1) * 512], ps[:], AF.Relu,
                    bias=zero_sb, accum_out=su_part[:, j:j + 1],
                )
            else:
                nc.vector.tensor_scalar(
                    u_sb[:, j * 512:(j + 1) * 512], ps[:], 0.0, 0.0,
                    op0=ALU.max, op1=ALU.add,
                    accum_out=su_part[:, j:j + 1],
                )
            if j < NJ - 1:
                # fill the DMA-receipt wait until chunk j+1 releases
                kdense(2)

        # ---- C^T coefficient matrix ------------------------------------
        # Ungated junk runs right after the last u matmul; the gated
        # keepalive picks up when the (vector-half) j3 evac lands.
        kdense(3)
        keepalive(u_sb[:, 1920:1921], dense=2)
        su = stats.tile([T, 1], f32)
        nc.vector.tensor_reduce(su[:], su_part[:], axis=AX.X, op=ALU.add)
        su_bf = stats.tile([T, 1], bf16)
        nc.gpsimd.tensor_copy(su_bf[:], su[:])
        q_sb = stats.tile([T, 1], bf16)
        nc.scalar.activation(q_sb[:], su[:], AF.Ln, scale=C2, bias=xdvec_sb)

        # ct[s,t] = exp(Q_{s-1} + trik[s,t]): the true C also carries a
        # exp(-Q_t) column factor, but that scales a* and v rows by a
        # positive per-row constant, which the downstream LayerNorms cancel
        # exactly. The exp(-Q_s) row factor of W is applied via wneg below.
        qc = p_ps.tile([T, T], f32, tag="cc", bufs=1)     # Q_{s-1} column
        nc.tensor.matmul(qc[:, 0:1], ustrict_sb, q_sb[:], start=True, stop=True)
        # Keepalives AFTER qc in the PE queue, gated on the already-ready
        # su_bf: they fill the serial colsc->ct stretch without pushing
        # qc (or anything else) later.
        keepalive(su_bf[:], dense=3)
        colsc = stats.tile([T, 1], f32)
        nc.vector.tensor_copy(colsc[:], qc[:, 0:1])
        ct_sb = work.tile([T, T], bf16)               # C^T [s,t]
        nc.scalar.activation(ct_sb[:], trik_sb[:], AF.Exp, bias=colsc[:])
        qcol = stats.tile([T, 1], f32)
        nc.vector.tensor_add(qcol[:], colsc[:], q_sb[:])
        wneg = stats.tile([T, 1], f32)                # exp(-Q_s)
        nc.scalar.activation(wneg[:], qcol[:], AF.Exp, scale=-1.0,
                             bias=zero_sb)
        cinv2 = stats.tile([T, 1], f32)               # exp(-2Q_s)
        nc.scalar.activation(cinv2[:], qcol[:], AF.Exp, scale=-2.0,
                             bias=zero_sb)
        q2neg = stats.tile([T, 1], f32)               # -2Q_s
        nc.vector.tensor_scalar_mul(q2neg[:], qcol[:], -2.0)

        # ---- X^T chunks = u_c @ C^T; G = X X^T, interleaved ------------
        xt_sb = work.tile([128, N], bf16)
        g = p_ps.tile([T, T], f32, tag="g", bufs=1)

        def xt_mm(cp):
            # two n-chunks share one PSUM tile -> one [128,256] evac
            tp = p_ps.tile([128, 2 * T], f32, tag="ch", bufs=3)
            for h in range(2):
                c = 2 * cp + h
                nc.tensor.matmul(tp[:, h * T:(h + 1) * T],
                                 u_sb[:, c * T:(c + 1) * T], ct_sb[:],
                                 start=True, stop=True)
            if cp % 2 == 0:
                nc.vector.tensor_copy(
                    xt_sb[:, 2 * cp * T:(2 * cp + 2) * T], tp[:])
            else:
                nc.scalar.copy(
                    xt_sb[:, 2 * cp * T:(2 * cp + 2) * T], tp[:])

        for cp in range(KN // 2):
            xt_mm(cp)
            if cp >= 2:
                for cg in (2 * (cp - 2), 2 * (cp - 2) + 1):
                    nc.tensor.matmul(g[:], xt_sb[:, cg * T:(cg + 1) * T],
                                     xt_sb[:, cg * T:(cg + 1) * T],
                                     start=(cg == 0), stop=False)
        for cg in range(KN - 4, KN):
            nc.tensor.matmul(g[:], xt_sb[:, cg * T:(cg + 1) * T],
                             xt_sb[:, cg * T:(cg + 1) * T],
                             start=False, stop=(cg == KN - 1))

        # ---- vn = LN(emb) ----------------------------------------------
        # Emitted AFTER xt/G: the Tile scheduler orders engine queues by
        # emission priority, and emitting vn earlier made its bn_stats
        # (waiting on the emb DMA piece) block the critical j3-evac/su
        # chain on the vector queue.
        vn_sb = work.tile([T, D], bf16)
        r_vn, n_vn = ln_stats(emb_sb, "vn")
        nc.gpsimd.tensor_scalar(vn_sb[:], emb_sb, r_vn[:], n_vn[:],
                                op0=ALU.mult, op1=ALU.add)

        # ---- a*^T = vn^T @ W directly (no LN, no transposes) -----------
        # a* has EXACTLY zero row-mean: vn = LN(emb) has zero row-sums and
        # a* = W^T @ vn, so mean_d a*[t,d] = sum_s W[s,t] * 0 = 0. And
        # relu(r*z) = r*relu(z) for the positive rstd r, so LN(a*) can be
        # skipped entirely: its rstd factor rides the v rows into the final
        # LayerNorm, which absorbs any positive per-row scale via the
        # nq/cinv2 correction. a^T comes straight from the PE as vn^T @ W
        # (lhsT = vn d-chunk), so yc starts ~3us earlier and the PE never
        # idles long enough for the HAM to re-throttle mid-kernel.
        wt_sb = work.tile([T, T], bf16)
        nc.vector.scalar_tensor_tensor(wt_sb[:], g[:], wneg[:], dmaskT_sb,
                                       op0=ALU.mult, op1=ALU.mult)
        kdense(2)  # cover the G->wt->atp latency
        atp = p_ps.tile([128, KD * T], f32, tag="ch", bufs=3)
        for k in range(KD):
            nc.tensor.matmul(atp[:, k * T:(k + 1) * T],
                             vn_sb[:, k * 128:(k + 1) * 128], wt_sb[:],
                             start=True, stop=True)
        # t-major copy feeds only the variance stats (for the final-LN eps
        # correction) — off the critical path.
        aps = p_ps.tile([T, D], f32, tag="g", bufs=1)
        nc.tensor.matmul(aps[:], wt_sb[:], vn_sb[:], start=True, stop=True)
        at_sb = work.tile([128, KD * T], bf16)
        nc.vector.tensor_copy(at_sb[:, 0:T], atp[:, 0:T])
        nc.vector.tensor_copy(at_sb[:, T:2 * T], atp[:, T:2 * T])
        kdense(2)  # cover the at-evac latency before yc starts

        # veps_a = var(a_true) + eps, lv_a = ln(veps_a); the v rows carry
        # scale exp(2Q_t)*sqrt(veps_a), so the final LN needs
        # nq_v = -2Q - lv_a/2 and cinv2_v = exp(2*nq_v). Emitted inside
        # the yc loop (mid-phase vector slack) via these thunks.
        stat6a = stats.tile([T, 6], f32)
        mva = stats.tile([T, 2], f32)
        vepsa = stats.tile([T, 1], f32)
        lva = stats.tile([T, 1], f32)
        nqv = stats.tile([T, 1], f32)
        cinv2v = stats.tile([T, 1], f32)

        def a_stats_1():
            nc.vector.bn_stats(stat6a[:], aps[:])
            nc.vector.bn_aggr(mva[:], stat6a[:])

        def a_stats_2():
            nc.vector.tensor_scalar(vepsa[:], mva[:, 1:2], cinv2[:], LN_EPS,
                                    op0=ALU.mult, op1=ALU.add)
            nc.scalar.activation(lva[:], vepsa[:], AF.Ln, bias=zero_sb)
            nc.vector.scalar_tensor_tensor(nqv[:], lva[:], -0.5, q2neg[:],
                                           op0=ALU.mult, op1=ALU.add)
            nc.scalar.activation(cinv2v[:], nqv[:], AF.Exp, scale=2.0,
                                 bias=zero_sb)

        # ---- Ycore^T chunks -> Y^T = relu(.)*X^T -> v accumulation -----
        # Chunk pairs share one PSUM tile on the (long-dead) pu banks, so
        # yc, xt ('ch') and the v accumulator rotate on disjoint banks —
        # six yc chunk-slots in flight instead of three.
        yt_sb = work.tile([128, N], bf16)
        vps = p_ps.tile([T, D], f32, tag="g", bufs=1)

        def yt_evac(c, yc_ap):
            # single fused relu*x on vector (~160ns) — vector paces the
            # phase just below the PE's ~215ns/chunk stream time
            nc.vector.scalar_tensor_tensor(
                yt_sb[:, c * T:(c + 1) * T], yc_ap, 0.0,
                xt_sb[:, c * T:(c + 1) * T], op0=ALU.max, op1=ALU.mult,
            )

        for cp in range(KN // 2):
            yc2 = p_ps.tile([128, 2 * T], f32, tag="pu", bufs=3)
            for h in range(2):
                c = 2 * cp + h
                for k in range(KD):
                    nc.tensor.matmul(
                        yc2[:, h * T:(h + 1) * T],
                        dyT_sb[:, c * 256 + k * 128: c * 256 + (k + 1) * 128],
                        at_sb[:, k * T:(k + 1) * T],
                        start=(k == 0), stop=(k == KD - 1),
                    )
                yt_evac(c, yc2[:, h * T:(h + 1) * T])
                if c >= 3:
                    cv = c - 3
                    nc.tensor.matmul(vps[:], yt_sb[:, cv * T:(cv + 1) * T],
                                     eT_sb[:, cv * D:(cv + 1) * D],
                                     start=(cv == 0), stop=False)
            if cp == 2 or cp == 5:
                # the phase runs ~67% real PE duty; two small junk chains
                # are enough to keep the HAM windows above threshold
                kdense(2)
            if cp == 3:
                a_stats_1()
            if cp == 5:
                a_stats_2()
        for cv in (KN - 3, KN - 2, KN - 1):
            nc.tensor.matmul(vps[:], yt_sb[:, cv * T:(cv + 1) * T],
                             eT_sb[:, cv * D:(cv + 1) * D],
                             start=False, stop=(cv == KN - 1))

        # ---- v* = LN(vps) -> out (bf16; host upcasts) ------------------
        # Row-split applies so each post follows its own engine's apply
        # with no cross-engine wait; posts go on the scalar and gpsimd
        # HWDGE queues (the sync queue's out-DMA receipt measured 1.95us
        # vs 0.25us on a lightly-used queue).
        r_v, n_v = ln_stats(vps[:], "vs", nq=nqv, cinv2=cinv2v)
        # Both apply halves on vector: a second engine's apply always
        # serializes behind the first anyway (conservative cross-engine
        # wait, measured), and two back-to-back vector ops are faster
        # than ACT + the serialization. One post on the scalar queue:
        # the second DMA on any queue pays ~2us extra.
        vstar_sb = work.tile([T, D], bf16)
        nc.vector.tensor_scalar(vstar_sb[:, 0:T], vps[:, 0:T],
                                r_v[:], n_v[:], op0=ALU.mult, op1=ALU.add)
        nc.vector.tensor_scalar(vstar_sb[:, T:D], vps[:, T:D],
                                r_v[:], n_v[:], op0=ALU.mult, op1=ALU.add)
        nc.scalar.dma_start(d_out[:, :], vstar_sb[:, :])

    # The const-AP pool memsets are the first *named* instructions and
    # nothing references them anymore (all activation biases are explicit
    # APs) — delete the dead code so the program truly starts at the DMA
    # posts.
    const_reads = set()
    for f in nc.m.functions:
        for bb in f.blocks:
            for ins in bb.instructions:
                for a in ins.ins:
                    n = getattr(a, "memref", "") or ""
                    if n.startswith("const-"):
                        const_reads.add((ins.name, n))
    assert not const_reads, f"const-AP pool still referenced: {const_reads}"
    for f in nc.m.functions:
        for bb in f.blocks:
            keep = []
            for ins in bb.instructions:
                if ins.opcode == "Memset" and any(
                    (getattr(a, "memref", "") or "").startswith("const-")
                    for a in ins.outs
                ):
                    continue
                keep.append(ins)
            bb.instructions = keep
    return _split_multiwait(nc, mybir)


def _numpy_fallback(embeddings, E, Dx, Dy, x_state, rho_state):
    # General-path reference (only used if initial states are nonzero).
    def ln(x):
        m = x.mean(-1, keepdims=True)
        v = ((x - m) ** 2).mean(-1, keepdims=True)
        return (x - m) / np.sqrt(v + LN_EPS)

    x_s = x_state.astype(np.float32).copy()
    rho = rho_state.astype(np.float32).copy()
    outs = np.zeros((B, T, D), dtype=np.float32)
    for t in range(T):
        v_prev = embeddings[:, t, :]
        x_upd = np.maximum(v_prev @ Dx.T, 0.0)
        x_t = XD * x_s + x_upd
        x_t = x_t / np.maximum(np.abs(x_t).sum(-1, keepdims=True), 1e-12)
        a_star = np.einsum("bdn,bn->bd", rho, x_t)
        y_core = ln(a_star) @ Dy.T
        y_t = np.maximum(y_core, 0.0) * np.maximum(x_t, 0.0)
        outs[:, t, :] = ln(y_t @ E.T)
        vn = ln(v_prev)
        rho = UD * rho + np.einsum("bd,bn->bdn", vn, x_t)
        x_s = x_t
    return outs


def kernel(embeddings, E, Dx, Dy, x_state, rho_state):
    import ml_dtypes

    embeddings = np.ascontiguousarray(embeddings, dtype=np.float32)
    E = np.ascontiguousarray(E, dtype=np.float32)
    Dx = np.ascontiguousarray(Dx, dtype=np.float32)
    Dy = np.ascontiguousarray(Dy, dtype=np.float32)

    if np.any(x_state) or np.any(rho_state):
        return _numpy_fallback(embeddings, E, Dx, Dy,
                               np.asarray(x_state, np.float32),
                               np.asarray(rho_state, np.float32))

    from concourse.bass_utils import run_bass_kernel_spmd

    if "nc" not in _cache:
        _cache["nc"] = _build()
    nc = _cache["nc"]

    bf = ml_dtypes.bfloat16
    consts_bf = _consts_bf16()
    cf32 = _consts_f32()
    # SBUF-layout packing: row p holds that partition's contiguous span.
    dxT = np.ascontiguousarray(
        Dx.T.reshape(KD, 128, NJ, 512).transpose(1, 2, 0, 3).reshape(128, KD * N)
    ).astype(bf)
    dyT = np.ascontiguousarray(
        Dy.T.reshape(KD, 128, KN, 128).transpose(1, 2, 0, 3).reshape(128, KD * N)
    ).astype(bf)
    eT = np.ascontiguousarray(
        E.T.reshape(KN, 128, D).transpose(1, 0, 2).reshape(128, KN * D)
    ).astype(bf)

    in_maps = []
    for b in range(B):
        emb_b = embeddings[b]
        embT_b = np.ascontiguousarray(
            emb_b.T.reshape(KD, 128, T).transpose(1, 0, 2).reshape(128, KD * T)
        ).astype(bf)
        blob = np.concatenate(
            [embT_b, dxT, consts_bf, emb_b.astype(bf), dyT, eT], axis=1)
        assert blob.shape == (128, CB), blob.shape
        in_maps.append({"blob": np.ascontiguousarray(blob), "cf32": cf32})

    res = run_bass_kernel_spmd(nc, in_maps, list(range(B)))
    _cache["last_results"] = res
    return np.stack([np.asarray(res.results[i]["out"], dtype=np.float32)
                     for i in range(B)])

